# revision 2
# baseline (speedup 1.0000x reference)
"""v5: v4 + fp8 DoubleRow FC.

Wfc ships fp8e4 x32; the softmax-normalize multiply writes fp8e4 "ofull8"
scaled x16 (sel carries 1/16 so rsc = 16/S); the FC contracts K=256 per
DoubleRow matmul and the relu activation rescales by 1/512 (exact -- relu
is positively homogeneous).

Host ships q/k/v and Wq/Wk/Wv in fp8e4 (weights pre-scaled x32 so they sit
in e4m3's normal range); projections contract K=256 per DoubleRow matmul
(half the matmuls, half the input DMA bytes). qh/kh come out x32 (bf16
holds them fine); the exp scale absorbs the 32x32 factor; the v drain
rescales by 1/32 while converting PSUM -> fp8e4.

v3 base: fp8 AV + pair-exp on top of v2's scheduling.

vs v2 (HW-verified building blocks, rel tolerance 2e-2):
- AV contraction in fp8 DoubleRow: one K=256 matmul per key-block-pair
  (306ns HW vs 550ns for the bf16 row-tiled pair scheme). Weights are the
  projected v in fp8e4, padded to 128 columns (ISA requires 64/128): cols
  0:64 = v, col 64 = ones (softmax sums ride along), 65:128 = zero.
- exp emits fp8e5 pairs [128, 2, 512] in ONE instruction per block-pair,
  alternating ACT (LUT exp, 1097ns HW) / DVE (Schraudolph e5m2 bit-trick
  via int8 tensor_scalar, 1217ns HW), reading the score pair from a
  two-bank PSUM tile written by the (T0,T8) bf16 score matmuls.
  exp is shifted: pt = exp(s/8 - 2) -- cancels in softmax, keeps e5m2 in
  range (max score ~4.5 sigma -> e^2.5 = 12; negatives stay positive bits).
- Single av accumulator [128, 512] per qc -> drain is one ACT copy of
  [65, 512] (numerators + sums row) -> one A2A shard DMA per qc.
- PSUM: four 4KB two-bank slots (tag psp). v/k projection accumulators are
  packed two-banks-per-slot; FC uses [P, 2, 512] pf pairs (both nm halves
  of an sb), epilogue relu/add/store in [P, 1024] ops.
"""

import os
import sys

sys.path.insert(0, "/opt/trn_rl_repo")

import numpy as np
import ml_dtypes

import concourse.bass as bass
import concourse.mybir as mybir
import concourse.tile as tile
from concourse import bacc
from concourse.bass_utils import run_bass_kernel_spmd

SEQ = 4096
DM = 1024
NH = 16
DK = 64
DV = 64
CORES = 8
P = 128
HL = 2 * DK
SROWS = SEQ // CORES
MO = DM // P
F32 = mybir.dt.float32
BF16 = mybir.dt.bfloat16
F8E4 = mybir.dt.float8e4
F8E5 = mybir.dt.float8e5

AV_LEAD = int(os.environ.get("AV_LEAD", "2"))
ACT_SHARE = int(os.environ.get("ACT_SHARE", "8"))  # of 16 pair-exps per qc

LOG2E = float(np.log2(np.e))
SCALE = 1.0 / (8.0 * 32.0 * 32.0)  # 1/sqrt(DK), weights x32 each side
SHIFT = 0.0  # no shift: scores reach +-9.4; e5m2 bits must stay in [0,124)
A_E5 = 4.0 * LOG2E * SCALE
# HW DVE float->int8 conversion rounds to nearest; -0.225 centers the
# Schraudolph sawtooth (calibrated in CoreSim where conversion truncates,
# so sim shows a ~0.92x mean ratio -- harmless, cancels in softmax).
B_E5 = 60.0 - SHIFT * 4.0 * LOG2E - 0.225


def build(seq=SEQ):
    srows = seq // CORES
    kb = seq // P
    qcw = min(512, seq)
    qcs = seq // qcw
    sb_blocks = srows // P

    nc = bacc.Bacc(
        "TRN2",
        target_bir_lowering=False,
        debug=False,
        enable_asserts=True,
        num_devices=CORES,
    )

    qT = nc.dram_tensor("qT", [DM, seq], F8E4, kind="ExternalInput").ap()
    kT = nc.dram_tensor("kT", [DM, seq], F8E4, kind="ExternalInput").ap()
    vT = nc.dram_tensor("vT", [DM, seq], F8E4, kind="ExternalInput").ap()
    wqT = nc.dram_tensor("wqT", [DM, HL], F8E4, kind="ExternalInput").ap()
    wkT = nc.dram_tensor("wkT", [DM, HL], F8E4, kind="ExternalInput").ap()
    wvT = nc.dram_tensor("wvT", [DM, HL], F8E4, kind="ExternalInput").ap()
    wfcT = nc.dram_tensor("wfcT", [DM, DM], F8E4, kind="ExternalInput").ap()
    qres = nc.dram_tensor("qres", [srows, DM], F32, kind="ExternalInput").ap()
    sel_in = nc.dram_tensor("sel", [2, P], BF16, kind="ExternalInput").ap()
    out = nc.dram_tensor("out", [srows, DM], F32, kind="ExternalOutput").ap()

    qT_r = qT.rearrange("(o p) s -> p o s", p=P)
    kT_r = kT.rearrange("(o p) s -> p o s", p=P)
    vT_r = vT.rearrange("(o p) s -> p o s", p=P)
    wqT_r = wqT.rearrange("(o p) h -> p o h", p=P)
    wkT_r = wkT.rearrange("(o p) h -> p o h", p=P)
    wvT_r = wvT.rearrange("(o p) h -> p o h", p=P)
    wfcT_r = wfcT.rearrange("(o p) d -> p o d", p=P)
    qres_r = qres.rearrange("(b p) d -> p b d", p=P)
    out_r = out.rearrange("(b p) d -> p b d", p=P)

    with tile.TileContext(nc) as tc:
        with (
            tc.tile_pool(name="const", bufs=1) as cpool,
            tc.tile_pool(name="xin", bufs=8) as xpool,
            tc.tile_pool(name="pt", bufs=6) as ptpool,
            tc.tile_pool(name="small", bufs=3) as spool,
            tc.tile_pool(name="smp", bufs=1) as smpool,
            tc.tile_pool(name="psp", bufs=4, space="PSUM") as psp,
            tc.tile_pool(name="dram", bufs=1, space="DRAM") as dr,
        ):
            # ---- weights ----
            wv_sb = cpool.tile([P, MO, HL], F8E4, tag="wv")
            wk_sb = cpool.tile([P, MO, HL], F8E4, tag="wk")
            wq_sb = cpool.tile([P, MO, HL], F8E4, tag="wq")
            for j in range(4):
                nc.scalar.dma_start(
                    wv_sb[:, 2 * j : 2 * j + 2, :], wvT_r[:, 2 * j : 2 * j + 2, :]
                )
            for j in range(4):
                nc.scalar.dma_start(
                    wk_sb[:, 2 * j : 2 * j + 2, :], wkT_r[:, 2 * j : 2 * j + 2, :]
                )
            for j in range(4):
                nc.scalar.dma_start(
                    wq_sb[:, 2 * j : 2 * j + 2, :], wqT_r[:, 2 * j : 2 * j + 2, :]
                )
            sel = cpool.tile([2, P], BF16, tag="sel")
            nc.sync.dma_start(sel[:], sel_in[:])
            bias_t = cpool.tile([P, 1], F32, tag="bias")
            nc.vector.memset(bias_t[:], -SHIFT)

            qh2 = cpool.tile([P, seq], BF16, tag="qh2")
            qhs = cpool.tile([P, seq], BF16, tag="qhs")
            kh2 = cpool.tile([P, seq], BF16, tag="kh2")
            khs = cpool.tile([P, seq], BF16, tag="khs")
            # fp8e4 projected v, padded: [v(64) | ones | 0*63] per key block
            vh8 = [
                cpool.tile([P, kb, P], F8E4, tag=f"vh{h}", name=f"vh{h}")
                for h in range(2)
            ]
            for h in range(2):
                nc.vector.memset(vh8[h][:, :, DV:], 0.0)
                nc.vector.memset(vh8[h][:, :, DV : DV + 1], 1.0)
            outT65 = [
                cpool.tile([DV + 1, seq], BF16, tag=f"oT{h}", name=f"oT{h}")
                for h in range(2)
            ]

            engs = [nc.sync, nc.gpsimd, nc.scalar]

            def load_chunks(src_r):
                """fp8 chunk pairs: xts[g][:, j, :] holds chunk 2g+j."""
                xts = []
                for g in range(MO // 2):
                    xt = xpool.tile([P, 2, seq], F8E4, tag="xin",
                                    name=f"xin{g}")
                    for j in range(2):
                        o = 2 * g + j
                        if o == 0:
                            for q4 in range(4):
                                engs[q4 % 2].dma_start(
                                    xt[32 * q4 : 32 * (q4 + 1), j, :],
                                    src_r[32 * q4 : 32 * (q4 + 1), o, :],
                                )
                        else:
                            engs[o % 3].dma_start(xt[:, j, :], src_r[:, o, :])
                    xts.append(xt)
                return xts

            # ---- V proj: 8 accumulators packed per two-bank slot ----
            xts = load_chunks(vT_r)
            pvq = [
                psp.tile([P, 8, HL], F32, tag="psp", name=f"pvq{g}")
                for g in range(kb // 8)
            ]
            for g in range(MO // 2):
                for b in range(kb):
                    nc.tensor.matmul(
                        pvq[b // 8][:, b % 8, :],
                        xts[g][:, :, b * P : (b + 1) * P],
                        wv_sb[:, 2 * g : 2 * g + 2, :],
                        start=(g == 0 and b % 4 == 0),
                        stop=(g == MO // 2 - 1),
                        skip_group_check=(b % 4 != 0),
                        perf_mode=mybir.MatmulPerfMode.DoubleRow,
                    )
            for g in range(kb // 8):
                gs = slice(8 * g, 8 * g + 8)
                if g % 2 == 0:
                    nc.scalar.activation(
                        out=vh8[0][:, gs, :DV], in_=pvq[g][:, :, :DK],
                        func=mybir.ActivationFunctionType.Copy, scale=1.0 / 32.0)
                    nc.vector.tensor_scalar_mul(
                        out=vh8[1][:, gs, :DV], in0=pvq[g][:, :, DK:HL],
                        scalar1=1.0 / 32.0)
                else:
                    nc.vector.tensor_scalar_mul(
                        out=vh8[0][:, gs, :DV], in0=pvq[g][:, :, :DK],
                        scalar1=1.0 / 32.0)
                    nc.scalar.activation(
                        out=vh8[1][:, gs, :DV], in_=pvq[g][:, :, DK:HL],
                        func=mybir.ActivationFunctionType.Copy, scale=1.0 / 32.0)

            # ---- K proj: two 512-col groups per two-bank slot ----
            xts = load_chunks(kT_r)
            pgroups = seq // 512
            pps = [
                psp.tile([P, 2, 512], F32, tag="psp", name=f"ppk{g2}")
                for g2 in range(pgroups // 2)
            ]
            for gg in range(MO // 2):
                for g in range(pgroups):
                    nc.tensor.matmul(
                        pps[g // 2][:HL, g % 2, :],
                        wk_sb[:, 2 * gg : 2 * gg + 2, :],
                        xts[gg][:, :, g * 512 :][:, :, :512],
                        start=(gg == 0),
                        stop=(gg == MO // 2 - 1),
                        skip_group_check=(g % 2 == 1),
                        perf_mode=mybir.MatmulPerfMode.DoubleRow,
                    )
            for g2 in range(pgroups // 2):
                gsl = slice(g2 * 1024, (g2 + 1) * 1024)
                nc.scalar.copy(out=kh2[:, gsl], in_=pps[g2][:HL])
                nc.vector.tensor_copy(out=khs[0:DK, gsl], in_=pps[g2][DK:HL])
                nc.vector.tensor_copy(out=khs[DK:HL, gsl], in_=pps[g2][0:DK])

            # ---- Q proj (baseline-style, fully before attention) ----
            xts = load_chunks(qT_r)
            ppq = [
                psp.tile([P, 2, 512], F32, tag="psp", name=f"ppq{g2}")
                for g2 in range(pgroups // 2)
            ]
            for gg in range(MO // 2):
                for g in range(pgroups):
                    nc.tensor.matmul(
                        ppq[g // 2][:HL, g % 2, :],
                        wq_sb[:, 2 * gg : 2 * gg + 2, :],
                        xts[gg][:, :, g * 512 :][:, :, :512],
                        start=(gg == 0),
                        stop=(gg == MO // 2 - 1),
                        skip_group_check=(g % 2 == 1),
                        perf_mode=mybir.MatmulPerfMode.DoubleRow,
                    )
            for g2 in range(pgroups // 2):
                gsl = slice(g2 * 1024, (g2 + 1) * 1024)
                nc.scalar.copy(out=qh2[:, gsl], in_=ppq[g2][:HL])
                nc.vector.tensor_copy(out=qhs[0:DK, gsl], in_=ppq[g2][DK:HL])
                nc.vector.tensor_copy(out=qhs[DK:HL, gsl], in_=ppq[g2][0:DK])

            # late constants on the cheap-dispatch gpsimd queue
            wfc_sb = cpool.tile([P, MO, DM], F8E4, tag="wfc")
            for j in range(4):
                nc.gpsimd.dma_start(
                    wfc_sb[:, 2 * j : 2 * j + 2, :], wfcT_r[:, 2 * j : 2 * j + 2, :]
                )

            # ---- attention ----
            a2a_in, a2a_out = [], []
            hchunks = (CORES * DK) // P
            ofull, recips = [None, None], [None, None]
            qres_sb = {}
            dvis = [
                dr.tile([CORES * (DV + 1), srows], BF16, name=f"a2ai{h}")
                for h in range(2)
            ]
            dvos = [
                dr.tile([CORES * (DV + 1), srows], BF16, name=f"a2ao{h}")
                for h in range(2)
            ]

            def _fc_load(h):
                of = cpool.tile([P, hchunks, srows], BF16, tag=f"of{h}",
                                name=f"of{h}")
                rc = smpool.tile([2, hchunks, srows], BF16, tag=f"rc{h}",
                                 name=f"rc{h}")
                dvo = dvos[h]
                fengs = [nc.sync, nc.gpsimd, nc.scalar]
                for o in range(hchunks):
                    for g in range(2):
                        j = 2 * o + g
                        fengs[j % 3].dma_start(
                            of[DK * g : DK * (g + 1), o, :],
                            dvo[(DV + 1) * j : (DV + 1) * j + DV, :],
                        )
                        fengs[(j + 1) % 3].dma_start(
                            rc[g : g + 1, o, :],
                            dvo[(DV + 1) * j + DV : (DV + 1) * (j + 1), :],
                        )
                ofull[h] = of
                recips[h] = rc

            of8s = [
                cpool.tile([P, hchunks, srows], F8E4, tag=f"of8_{h}",
                           name=f"of8_{h}")
                for h in range(2)
            ]

            def _fc_scale(h, sel_t=None):
                # sel carries 1/16 -> rsc = 16/S; ofull8 = num * 16/S (e4m3)
                sel_t = sel if sel_t is None else sel_t
                for o in range(hchunks):
                    bc = psp.tile([P, srows], F32, tag="psp", name=f"bc{h}{o}")
                    nc.tensor.matmul(
                        bc[:], sel_t[:], recips[h][:, o, :], start=True, stop=True
                    )
                    rsc = spool.tile([P, srows], F32, tag="rsc", bufs=2,
                                     name=f"rsc{h}{o}")
                    nc.vector.reciprocal_approx_fast(out=rsc[:], in_=bc[:])
                    nc.vector.tensor_mul(
                        out=of8s[h][:, o, :], in0=ofull[h][:, o, :], in1=rsc[:]
                    )

            for h in range(2):
                klo, qlo = (kh2, qh2) if h == 0 else (khs, qhs)
                khi, qhi = (khs, qhs) if h == 0 else (kh2, qh2)
                pend_drain = None
                for qc in range(qcs):
                    q0 = qc * qcw
                    av = psp.tile([P, qcw], F32, tag="psp", name=f"av{h}_{qc}")
                    pts = {}
                    nbp = kb // 2
                    for step in range(nbp + AV_LEAD):
                        if step == 1 and pend_drain is not None:
                            pend_drain()
                            pend_drain = None
                        if step < nbp:
                            bp = step
                            b0, b1 = 2 * bp, 2 * bp + 1
                            scp = psp.tile([P, 2, qcw], F32, tag="psp",
                                           name=f"scp{h}_{qc}_{bp}")
                            nc.tensor.matmul(
                                scp[:, 0, :],
                                klo[0:DK, b0 * P : (b0 + 1) * P],
                                qlo[0:DK, q0 : q0 + qcw],
                                start=True,
                                stop=True,
                                tile_position=(0, 0),
                            )
                            nc.tensor.matmul(
                                scp[:, 1, :],
                                khi[DK:HL, b1 * P : (b1 + 1) * P],
                                qhi[DK:HL, q0 : q0 + qcw],
                                start=True,
                                stop=True,
                                tile_position=(64, 0),
                                skip_group_check=True,
                            )
                            pt = ptpool.tile([P, 2, qcw], F8E5, tag="pt",
                                             name=f"pt{h}_{qc}_{bp}")
                            if (bp * ACT_SHARE) % 16 < ACT_SHARE:
                                nc.scalar.activation(
                                    out=pt[:],
                                    in_=scp[:],
                                    func=mybir.ActivationFunctionType.Exp,
                                    scale=SCALE,
                                )
                            else:
                                nc.vector.tensor_scalar(
                                    out=pt[:].bitcast(mybir.dt.int8),
                                    in0=scp[:],
                                    scalar1=A_E5,
                                    scalar2=B_E5,
                                    op0=mybir.AluOpType.mult,
                                    op1=mybir.AluOpType.add,
                                )
                            pts[bp] = pt
                        if step >= AV_LEAD:
                            bp2 = step - AV_LEAD
                            pt = pts.pop(bp2)
                            nc.tensor.matmul(
                                av[:],
                                vh8[h][:, 2 * bp2 : 2 * bp2 + 2, :],
                                pt[:, :, :],
                                start=(bp2 == 0),
                                stop=(bp2 == nbp - 1),
                                perf_mode=mybir.MatmulPerfMode.DoubleRow,
                            )

                    def _drain(av=av, h=h, qc=qc, q0=q0):
                        nc.scalar.copy(
                            out=outT65[h][:, q0 : q0 + qcw], in_=av[0 : DV + 1, :]
                        )
                    if qc == qcs - 1:
                        _drain()
                    else:
                        pend_drain = _drain
                    if h == 1 and qc == qcs - 3:
                        _fc_load(0)
                    if h == 0 and qc == qcs - 2:
                        for sb in range(sb_blocks):
                            qre = spool.tile([P, DM], F32, tag="qre",
                                             bufs=sb_blocks, name=f"qre{sb}")
                            nc.gpsimd.dma_start(qre[:], qres_r[:, sb, :])
                            qres_sb[sb] = qre
                # ship all shards just before the collective (subtile deps on
                # outT65 gate each shard DMA on its own qc drain; issuing
                # them inside the drains races with the collective's
                # counting-semaphore wait)
                for j in range(qcs):
                    nc.sync.dma_start(
                        dvis[h][(DV + 1) * j : (DV + 1) * (j + 1), :],
                        outT65[h][:, j * qcw : (j + 1) * qcw],
                    )
                nc.gpsimd.collective_compute(
                    "AllToAll",
                    mybir.AluOpType.bypass,
                    replica_groups=[list(range(CORES))],
                    ins=[dvis[h].opt()],
                    outs=[dvos[h].opt()],
                )
                a2a_in.append(dvis[h])
                a2a_out.append(dvos[h])

            # ---- FC + epilogue ----
            tok = spool.tile([1, 4], BF16, tag="tok", bufs=1)
            nc.sync.dma_start(
                tok[:],
                a2a_in[1][(DV + 1) * (CORES - 1) : (DV + 1) * (CORES - 1) + 1, 0:4],
            )
            sel2 = spool.tile([2, P], BF16, tag="sel2", bufs=1)
            nc.vector.tensor_copy(out=sel2[:], in_=sel[:])
            nc.vector.tensor_scalar(
                out=sel2[0:1, 0:4],
                in0=tok[:],
                scalar1=0.0,
                scalar2=1.0 / 16.0,
                op0=mybir.AluOpType.mult,
                op1=mybir.AluOpType.add,
            )
            pfs_all = {}

            def _fc_passA(sb):
                pf = psp.tile([P, 2, 512], F32, tag="psp", name=f"pf{sb}")
                pfs_all[sb] = pf
                for nm in range(2):
                    for u in range(hchunks // 2):
                        nc.tensor.matmul(
                            pf[:, nm, :],
                            of8s[0][:, 2 * u : 2 * u + 2, sb * P : (sb + 1) * P],
                            wfc_sb[:, 2 * u : 2 * u + 2, nm * 512 : (nm + 1) * 512],
                            start=(u == 0),
                            stop=False,
                            skip_group_check=(nm == 1),
                            perf_mode=mybir.MatmulPerfMode.DoubleRow,
                        )

            def _fc_passB_epi(sb):
                pf = pfs_all[sb]
                for nm in range(2):
                    for u in range(hchunks // 2):
                        nc.tensor.matmul(
                            pf[:, nm, :],
                            of8s[1][:, 2 * u : 2 * u + 2, sb * P : (sb + 1) * P],
                            wfc_sb[:, hchunks + 2 * u : hchunks + 2 * u + 2,
                                   nm * 512 : (nm + 1) * 512],
                            start=False,
                            stop=(u == hchunks // 2 - 1),
                            skip_group_check=(nm == 1),
                            perf_mode=mybir.MatmulPerfMode.DoubleRow,
                        )
                eo = spool.tile([P, DM], F32, tag="eo")
                nc.scalar.activation(
                    out=eo[:], in_=pf[:, :, :],
                    func=mybir.ActivationFunctionType.Relu,
                    scale=1.0 / 512.0,
                )
                nc.vector.tensor_add(out=eo[:], in0=eo[:], in1=qres_sb[sb][:])
                oeng = [nc.sync, nc.gpsimd, nc.scalar][sb % 3]
                oeng.dma_start(out_r[:, sb, :], eo[:])

            _fc_scale(0, sel2)
            for sb in (0, 1, 2):
                _fc_passA(sb)
            _fc_load(1)
            _fc_scale(1)
            _fc_passA(3)
            for sb in range(sb_blocks):
                _fc_passB_epi(sb)

    nc.compile()
    return nc


def _fc_perm():
    perm = []
    for h in range(2):
        for o in range(4):
            for p in range(P):
                perm.append(128 * (2 * o + p // 64) + h * 64 + (p % 64))
    return np.array(perm)


def make_in_maps(q, k, v, Wq, Wk, Wv, Wfc, seq=SEQ):
    srows = seq // CORES
    bf = ml_dtypes.bfloat16
    e4 = ml_dtypes.float8_e4m3
    qT = np.ascontiguousarray(q.T).astype(e4)
    kT = np.ascontiguousarray(k.T).astype(e4)
    vT = np.ascontiguousarray(v.T).astype(e4)
    wfcT = np.ascontiguousarray(Wfc.T[_fc_perm()] * 32.0).astype(e4)
    sel = np.zeros((2, P), bf)
    sel[0, :DK] = 1.0 / 16.0
    sel[1, DK:] = 1.0 / 16.0
    in_maps = []
    for c in range(CORES):
        sl = slice(c * HL, (c + 1) * HL)
        in_maps.append(
            {
                "qT": qT,
                "kT": kT,
                "vT": vT,
                "wqT": np.ascontiguousarray(Wq[sl].T * 32.0).astype(e4),
                "wkT": np.ascontiguousarray(Wk[sl].T * 32.0).astype(e4),
                "wvT": np.ascontiguousarray(Wv[sl].T * 32.0).astype(e4),
                "wfcT": wfcT,
                "sel": sel,
                "qres": np.ascontiguousarray(q[c * srows : (c + 1) * srows]).astype(
                    np.float32
                ),
            }
        )
    return in_maps


_NC_CACHE = {}


def kernel(q, k, v, Wq, Wk, Wv, Wfc):
    key = "full"
    if key not in _NC_CACHE:
        _NC_CACHE[key] = build()
    nc = _NC_CACHE[key]
    in_maps = make_in_maps(q, k, v, Wq, Wk, Wv, Wfc)
    trace = bool(int(os.environ.get("KERNEL_TRACE", "0")))
    tc_env = os.environ.get("KERNEL_TRACE_CORES", "")
    kw = {}
    if tc_env:
        kw["trace_cores"] = [int(x) for x in tc_env.split(",")]
    res = run_bass_kernel_spmd(nc, in_maps, list(range(CORES)), trace=trace, **kw)
    if trace:
        kernel.last_exec_time_ns = res.exec_time_ns
        kernel.last_profile = res
    out = np.concatenate([res.results[c]["out"] for c in range(CORES)], axis=0)
    return out.astype(np.float32)
